# revision 1
# baseline (speedup 1.0000x reference)
"""InvariantMessagePasser Bass kernel for 8 Trainium2 NeuronCores.

Algorithm
---------
Host: sort edges by center atom, partition atoms (and their contiguous
edge runs) across 8 cores balanced by edge count, and pack each core's
edges into 128-edge tiles holding at most 8 distinct center atoms
("slots"). Atom runs may split across tiles; the host accumulates
(tile, slot) partial results into the final per-atom output.

Device (per core, SPMD): per 128-edge tile,
  - gather neighbor embeddings with indirect DMA,
  - DVE: one-hot(slot) from an iota compare; per-l "ohsh" = one-hot x sh
    (lhsT of the scatter matmul, columns = (slot, m) pairs); t = rb * emb,
  - PE: per (l, subtile) matmul psum[(slot,m), k] += ohsh^T @ t_l  --
    this performs the segment scatter-add over the tile's edges,
  - ACT: psum -> sbuf copy; DMA the per-l staging blocks out.

Outputs land as out_l[macro, 8*m_l, S*32] staging blocks which the host
scatter-adds (np.add.at) into dens[n_atoms, 16, 32].

MP_SCALING is folded into sh on the host.
"""
import sys
import numpy as np

sys.path.insert(0, "/opt/trn_rl_repo")

import concourse.bass as bass            # noqa: E402
import concourse.bacc as bacc            # noqa: E402
import concourse.mybir as mybir          # noqa: E402
import concourse.tile as tile            # noqa: E402
from concourse.bass_utils import run_bass_kernel_spmd  # noqa: E402

L_MAX = 3
K = 32
NR = 16          # sum of (2l+1)
NL = 4
P = 128          # edges per tile (matmul contraction)
WA = 8           # atom slots per tile
S = 16           # subtiles (tiles) per macro
ML = [2 * l + 1 for l in range(NL)]          # [1,3,5,7]
R0 = [l * l for l in range(NL)]              # [0,1,4,9]
N_CORES = 8
MP_SCALING = 0.1

_prog_cache: dict = {}


def _build_program(M: int, n_atoms: int):
    """Build + compile the SPMD Bass program for M macros per core."""
    nc = bacc.Bacc("TRN2", target_bir_lowering=False, debug=False,
                   num_devices=N_CORES)
    f32 = mybir.dt.float32
    d_sh = nc.dram_tensor("sh_t", [M, P, S * NR], f32, kind="ExternalInput")
    d_rb = nc.dram_tensor("rb_t", [M, P, S * NL * K], f32, kind="ExternalInput")
    d_slot = nc.dram_tensor("slot_t", [M, P, S], f32, kind="ExternalInput")
    d_nbr = nc.dram_tensor("nbr_t", [M, P, S], mybir.dt.int32, kind="ExternalInput")
    d_emb = nc.dram_tensor("emb", [n_atoms, K], f32, kind="ExternalInput")
    d_iota = nc.dram_tensor("iota", [P, WA], f32, kind="ExternalInput")
    d_out = [nc.dram_tensor(f"out{l}", [M, WA * ML[l], S * K], f32,
                            kind="ExternalOutput") for l in range(NL)]

    with tile.TileContext(nc) as tc:
        with (
            tc.tile_pool(name="inp", bufs=3) as inp,
            tc.tile_pool(name="work", bufs=2) as work,
            tc.tile_pool(name="stage", bufs=2) as stpool,
            tc.tile_pool(name="psum", bufs=2, space="PSUM") as pp,
            tc.tile_pool(name="const", bufs=1) as cpool,
        ):
            iota_sb = cpool.tile([P, WA], f32)
            nc.sync.dma_start(iota_sb[:], d_iota[:])
            for m in range(M):
                sh_sb = inp.tile([P, S * NR], f32, tag="sh")
                nc.sync.dma_start(sh_sb[:], d_sh[m])
                rb_sb = inp.tile([P, S * NL * K], f32, tag="rb")
                nc.sync.dma_start(rb_sb[:], d_rb[m])
                slot_sb = inp.tile([P, S], f32, tag="slot")
                nc.sync.dma_start(slot_sb[:], d_slot[m])
                nbr_sb = inp.tile([P, S], mybir.dt.int32, tag="nbr")
                nc.sync.dma_start(nbr_sb[:], d_nbr[m])

                emb_sb = work.tile([P, S * K], f32, tag="emb")
                for s in range(S):
                    nc.gpsimd.indirect_dma_start(
                        out=emb_sb[:, s * K:(s + 1) * K],
                        out_offset=None,
                        in_=d_emb[:],
                        in_offset=bass.IndirectOffsetOnAxis(
                            ap=nbr_sb[:, s:s + 1], axis=0),
                    )

                oh_sb = work.tile([P, S * WA], f32, tag="oh")
                nc.vector.tensor_tensor(
                    out=oh_sb[:].rearrange("p (s a) -> p s a", s=S),
                    in0=slot_sb[:][:, :, None].to_broadcast([P, S, WA]),
                    in1=iota_sb[:][:, None, :].to_broadcast([P, S, WA]),
                    op=mybir.AluOpType.is_equal)

                oh_view = oh_sb[:].rearrange("p (s a) -> p s a", s=S)
                sh_view = sh_sb[:].rearrange("p (s r) -> p s r", s=S)
                ohsh_l = []
                for l in range(NL):
                    ml = ML[l]
                    osl = work.tile([P, S * WA * ml], f32, tag=f"ohsh{l}")
                    nc.vector.tensor_tensor(
                        out=osl[:].rearrange("p (s a r) -> p s a r", s=S, a=WA),
                        in0=oh_view[:, :, :, None].to_broadcast([P, S, WA, ml]),
                        in1=sh_view[:, :, None, R0[l]:R0[l] + ml]
                            .to_broadcast([P, S, WA, ml]),
                        op=mybir.AluOpType.mult)
                    ohsh_l.append(osl)

                t_sb = work.tile([P, S * NL * K], f32, tag="t")
                nc.vector.tensor_tensor(
                    out=t_sb[:].rearrange("p (s l k) -> p s l k", s=S, l=NL),
                    in0=rb_sb[:].rearrange("p (s l k) -> p s l k", s=S, l=NL),
                    in1=emb_sb[:].rearrange("p (s k) -> p s k", s=S)
                        [:, :, None, :].to_broadcast([P, S, NL, K]),
                    op=mybir.AluOpType.mult)

                t_view = t_sb[:].rearrange("p (s l k) -> p s l k", s=S, l=NL)
                for l in range(NL):
                    ml = ML[l]
                    ps = pp.tile([WA * ml, S * K], f32, tag=f"ps{l}")
                    for s in range(S):
                        nc.tensor.matmul(
                            out=ps[:, s * K:(s + 1) * K],
                            lhsT=ohsh_l[l][:, s * WA * ml:(s + 1) * WA * ml],
                            rhs=t_view[:, s, l, :],
                            start=True, stop=True)
                    st = stpool.tile([WA * ml, S * K], f32, tag=f"st{l}")
                    nc.scalar.copy(st[:], ps[:])
                    nc.sync.dma_start(d_out[l][m], st[:])
    nc.compile()
    return nc


def _pack_core(counts, atom_base):
    """Greedy-pack one core's (sorted) edges into tiles.

    counts: edges per atom for this core's atom range (local ids).
    Returns list of tiles; each tile is a list of (global_atom, take).
    """
    tiles = []
    cur: list = []
    used = 0
    for a_local, cnt in enumerate(counts):
        rem = int(cnt)
        while rem > 0:
            if used == P or len(cur) == WA:
                tiles.append(cur)
                cur = []
                used = 0
            take = min(rem, P - used)
            cur.append((atom_base + a_local, take))
            used += take
            rem -= take
    if cur:
        tiles.append(cur)
    return tiles


def _prep(sh, radial_basis, center_embedding, centers, neighbors, n_atoms):
    E = sh.shape[0]
    order = np.argsort(centers, kind="stable")
    c_sorted = centers[order]

    counts = np.bincount(c_sorted, minlength=n_atoms).astype(np.int64)
    cum = np.concatenate([[0], np.cumsum(counts)])
    # atom-range boundaries balanced by edge count
    bounds = [0]
    for c in range(1, N_CORES):
        bounds.append(int(np.searchsorted(cum, E * c // N_CORES)))
    bounds.append(n_atoms)

    core_tiles = []
    for c in range(N_CORES):
        a0, a1 = bounds[c], bounds[c + 1]
        core_tiles.append(_pack_core(counts[a0:a1], a0))
    T_core = [len(t) for t in core_tiles]
    M = (max(T_core) + S - 1) // S
    T_pad = M * S

    # per-(tile,position) edge index / slot / validity
    idx = np.zeros((N_CORES, T_pad, P), np.int64)
    pad = np.ones((N_CORES, T_pad, P), bool)
    slot = np.zeros((N_CORES, T_pad, P), np.float32)
    amap = np.full((N_CORES, T_pad, WA), -1, np.int64)
    for c in range(N_CORES):
        a0 = bounds[c]
        cursor = int(cum[a0])      # first sorted-edge index of this core
        for t, tl in enumerate(core_tiles[c]):
            used = 0
            for si, (atom, take) in enumerate(tl):
                idx[c, t, used:used + take] = np.arange(cursor, cursor + take)
                slot[c, t, used:used + take] = si
                amap[c, t, si] = atom
                cursor += take
                used += take
            pad[c, t, :used] = False

    # gather edge data into tile layout
    flat_idx = idx.reshape(-1)
    ordered = order[flat_idx]
    sh_g = (sh[ordered] * MP_SCALING).astype(np.float32)
    sh_g[pad.reshape(-1)] = 0.0
    rb_g = radial_basis.reshape(E, NL * K)[ordered].astype(np.float32)
    rb_g[pad.reshape(-1)] = 0.0
    nbr_g = neighbors[ordered].astype(np.int32)
    nbr_g[pad.reshape(-1)] = 0

    def to_macro(a, inner):
        # [C, T_pad, P, inner] -> [C, M, P, S*inner]
        a = a.reshape(N_CORES, M, S, P, inner)
        return np.ascontiguousarray(a.transpose(0, 1, 3, 2, 4)).reshape(
            N_CORES, M, P, S * inner)

    sh_t = to_macro(sh_g.reshape(N_CORES, T_pad, P, NR), NR)
    rb_t = to_macro(rb_g.reshape(N_CORES, T_pad, P, NL * K), NL * K)
    slot_t = to_macro(slot.reshape(N_CORES, T_pad, P, 1), 1)
    nbr_t = to_macro(nbr_g.reshape(N_CORES, T_pad, P, 1), 1)

    iota = np.broadcast_to(np.arange(WA, dtype=np.float32), (P, WA)).copy()
    emb = np.ascontiguousarray(center_embedding.astype(np.float32))

    in_maps = [{
        "sh_t": np.ascontiguousarray(sh_t[c]),
        "rb_t": np.ascontiguousarray(rb_t[c]),
        "slot_t": np.ascontiguousarray(slot_t[c]),
        "nbr_t": np.ascontiguousarray(nbr_t[c]),
        "emb": emb,
        "iota": iota,
    } for c in range(N_CORES)]
    return in_maps, amap, M


def _reassemble(results, amap, n_atoms):
    blocks = []
    for l in range(NL):
        ml = ML[l]
        acc = np.zeros((n_atoms, ml, K), np.float32)
        for c in range(N_CORES):
            arr = results[c][f"out{l}"]                     # [M, WA*ml, S*K]
            M = arr.shape[0]
            rows = arr.reshape(M, WA, ml, S, K).transpose(0, 3, 1, 2, 4)
            rows = rows.reshape(M * S * WA, ml, K)
            am = amap[c].reshape(-1)
            valid = am >= 0
            np.add.at(acc, am[valid], rows[valid])
        blocks.append(acc)
    return np.concatenate(blocks, axis=1)


def kernel(sh, radial_basis, center_embedding, centers, neighbors, n_atoms):
    n_atoms = int(n_atoms)
    in_maps, amap, M = _prep(np.asarray(sh), np.asarray(radial_basis),
                             np.asarray(center_embedding),
                             np.asarray(centers), np.asarray(neighbors),
                             n_atoms)
    key = (M, n_atoms)
    if key not in _prog_cache:
        _prog_cache[key] = _build_program(M, n_atoms)
    nc = _prog_cache[key]
    res = run_bass_kernel_spmd(nc, in_maps, list(range(N_CORES)))
    return _reassemble(res.results, amap, n_atoms)
